# revision 41
# baseline (speedup 1.0000x reference)
"""Trainium2 Bass kernel for GQA multi-head attention (nn_MultiHeadAttention).

Reference computation (fp32):
    q = h @ Wq^T -> RoPE ; k = h @ Wk^T -> RoPE ; v = h @ Wv^T
    scores = q k^T / sqrt(64) + causal_mask ; w = softmax(scores)
    out = (w v) @ Wo^T

Shapes: h [2,2048,2048], Wq [2048,2048], Wk/Wv [512,2048], Wo [2048,2048],
32 q heads / 8 kv heads (GQA group=4), head_dim 64.

Sharding: tensor-parallel over the 8 kv-head groups, one group per core.
Core g owns q heads [4g,4g+4), kv head g, Wo columns [256g, 256(g+1)).
Each core computes a full-token partial of the output projection; the host
sums the 8 partials (the Wo contraction splits over head blocks).

Structure: a single software-pipelined loop over the 8 token tiles of 512;
at step s it emits [hT prefetch for s+1] [attention for query tile s-1]
[QKV projection + RoPE for tile s] [Wo projection + output store for tile
s-2].  All weights are packed host-side so every DMA moves whole-partition
contiguous lines (128 fat descriptors instead of 2048 thin ones), and the
weight loads ride the scalar HWDGE ring while hT tiles ride the sync ring,
so the first matmul starts ~8us in instead of ~37us.  RoPE scratch is bf16
double-buffered; the rot_half partition swap runs per projection group so
the q/k tiles are ready group-by-group.  The Wo PSUM evacuations move to
DVE during exp-heavy steps (ACT is the attention-phase bottleneck).  The
last query tile's Wo projection is split into its two head-pair halves
written to separate DRAM partials (host adds them) so half of it overlaps
the final exp storm.
"""

import sys

for _p in ("/opt/trn_rl_repo",):
    if _p not in sys.path:
        sys.path.insert(0, _p)

import numpy as np
import ml_dtypes

D = 2048          # model dim
HD = 64           # head dim
S = 2048          # sequence
B = 2             # batch
T = B * S         # total tokens
EQ = 256          # q-projection rows per core (4 heads x 64)
TT = 512          # token tile (both projection and query tile)
NT = T // TT      # 8 merged steps
NDB = D // 128    # contraction blocks for projections
QT = 512          # query tile for attention
KBLK = 128        # key block for attention
BF16 = ml_dtypes.bfloat16

_CACHE = {}


def _build_program(causal: bool):
    """Build the single-core Bass/Tile program (identical across cores)."""
    import concourse.bass as bass
    import concourse.mybir as mybir
    import concourse.tile as tile
    from concourse import bacc
    from concourse.masks import make_identity

    f32 = mybir.dt.float32
    bf16 = mybir.dt.bfloat16

    nc = bacc.Bacc("TRN2", target_bir_lowering=False, debug=False)

    # hT packed host-side as [128, NT, 16, TT] so each (partition, tile)
    # line is one contiguous 16 KiB DMA descriptor
    hTP = nc.dram_tensor("hTP", [128, NT * NDB * TT], bf16, kind="ExternalInput").ap()
    # weights packed so partition lines are contiguous in DRAM
    wqP = nc.dram_tensor("wqP", [128, NDB * EQ], bf16, kind="ExternalInput").ap()
    wkvP = nc.dram_tensor("wkvP", [128, NDB * 2 * HD], bf16, kind="ExternalInput").ap()
    woP = nc.dram_tensor("woP", [128, 2 * D], bf16, kind="ExternalInput").ap()
    # deduplicated RoPE tables: [64, S] (the [128, T] SBUF tiles are built
    # by SBUF->SBUF copies -- rows 64:128 and the second batch repeat)
    cosD = nc.dram_tensor("cosD", [64, S], bf16, kind="ExternalInput").ap()
    # sin with rot_half sign AND partition swap pre-applied (see _host_inputs)
    sinD = nc.dram_tensor("sinD", [64, S], bf16, kind="ExternalInput").ap()
    # mask^T tiles, only used when causal=False
    maskT = nc.dram_tensor("maskT", [S, S], f32, kind="ExternalInput").ap()
    # outputs packed the same way: [128, NT, 16, TT] / [128, 16, QT]
    outP = nc.dram_tensor("outP", [128, NT * NDB * TT], bf16, kind="ExternalOutput").ap()
    # rp1 partial of the last query tile's Wo projection (host adds)
    outP2 = nc.dram_tensor("outP2", [128, NDB * QT], bf16, kind="ExternalOutput").ap()

    hT_b3 = hTP.rearrange("p (i n t) -> p i n t", i=NT, n=NDB)  # [128, 8, 16, TT]
    wqP_b = wqP.rearrange("p (n e) -> p n e", n=NDB)
    wkvP_b = wkvP.rearrange("p (n e) -> p n e", n=NDB)
    woP_b = woP.rearrange("p (n e) -> p n e", n=2)
    outP_b = outP.rearrange("p (i n t) -> p i n t", i=NT, n=NDB)  # [128, 8, 16, TT]
    outP2_b = outP2.rearrange("p (n t) -> p n t", n=NDB)          # [128, 16, QT]

    Exp = mybir.ActivationFunctionType.Exp
    PSUM = bass.MemorySpace.PSUM

    with tile.TileContext(nc) as tc:
        import contextlib

        with contextlib.ExitStack() as stack:
            const = stack.enter_context(tc.tile_pool(name="const", bufs=1))

            wq_s = const.tile([128, NDB, EQ], bf16)
            wkv_s = const.tile([128, NDB, 2 * HD], bf16)
            wo_s = const.tile([128, 2, D], bf16)
            # RoPE tables indexed mod S (both batches share positions)
            cos_s = const.tile([128, S], bf16)
            sinp_s = const.tile([128, S], bf16)
            # q tiles: 2-slot ring per head pair (attn reads slot (it)%2
            # while proj writes slot (it+1)%2)
            qt_s = [
                const.tile([128, 2, TT], bf16, tag=f"qt{i}", name=f"qt{i}")
                for i in range(2)
            ]
            kt_s = const.tile([128, T], bf16)
            va_s = const.tile([128, T // 128, HD + 1], bf16)
            tri_s = const.tile([128, 4, QT], bf16)
            ident = const.tile([128, 128], f32)
            ones64 = const.tile([1, 64], bf16)

            # weight loads on the scalar HWDGE ring (parallel with hT tiles
            # on the sync ring).  wq split in halves so the first projection
            # matmuls only wait for the first half.
            # strict need-order on ONE ring (single-ring FIFO = exact
            # bandwidth priority; ht tile 0 is emitted right here too, see
            # the loop below): wq first half, RoPE tables, ht0, the rest
            nc.scalar.dma_start(out=wq_s[:, 0:8, :], in_=wqP_b[:, 0:8, :])
            nc.scalar.dma_start(out=cos_s[0:64, :], in_=cosD)
            nc.scalar.dma_start(out=sinp_s[0:64, :], in_=sinD)

            make_identity(nc, ident)
            # ones column of the augmented V
            nc.gpsimd.memset(va_s[:, :, HD : HD + 1], 1.0)
            nc.gpsimd.memset(ones64, 1.0)
            # multiplicative causal masks for the 4 straddle offsets:
            # tri_s[p, j, f] = 1.0 where f >= p + 128*j else 0.0
            for j in range(4):
                nc.gpsimd.memset(tri_s[:, j, :], 1.0)
                nc.gpsimd.affine_select(
                    out=tri_s[:, j, :],
                    in_=tri_s[:, j, :],
                    compare_op=mybir.AluOpType.is_ge,
                    fill=0.0,
                    base=-128 * j,
                    channel_multiplier=-1,
                    pattern=[[1, QT]],
                )

            # ---------------- pools for the merged pipeline
            with contextlib.ExitStack() as pp:
                ht_pool = pp.enter_context(tc.tile_pool(name="ht", bufs=3))
                # m1 / z / swapped-z rope scratch (bf16, double-buffered)
                rp_pool = pp.enter_context(tc.tile_pool(name="rp", bufs=2))
                vs_pool = pp.enter_context(tc.tile_pool(name="vs", bufs=2))
                # shared-PSUM pool: proj accumulators, V transposes, Wo tiles
                ps_mm = pp.enter_context(
                    tc.tile_pool(name="ps_mm", bufs=2, space=PSUM)
                )
                ps_s = pp.enter_context(
                    tc.tile_pool(name="ps_s", bufs=2, space=PSUM)
                )
                ps_o = pp.enter_context(
                    tc.tile_pool(name="ps_o", bufs=1, space=PSUM)
                )
                pt_pool = pp.enter_context(tc.tile_pool(name="pt", bufs=6))
                on_pool = pp.enter_context(tc.tile_pool(name="on", bufs=2))
                nm_pool = pp.enter_context(tc.tile_pool(name="nm", bufs=1))
                oa_pool = pp.enter_context(tc.tile_pool(name="oa", bufs=1))
                dr_pool = pp.enter_context(
                    tc.tile_pool(name="dr", bufs=2, space="DRAM")
                )

                def prefetch(it, split=False, on_scalar=False):
                    htile = ht_pool.tile([128, NDB, TT], bf16, tag="ht",
                                         name=f"ht{it}")
                    eng = nc.scalar if (split or on_scalar) else nc.sync
                    if split:
                        eng.dma_start(
                            out=htile[:, 0:8, :], in_=hT_b3[:, it, 0:8, :]
                        )
                        eng.dma_start(
                            out=htile[:, 8:16, :], in_=hT_b3[:, it, 8:16, :]
                        )
                    else:
                        eng.dma_start(out=htile, in_=hT_b3[:, it, :, :])
                    return htile

                def proj_chunk(it, htile, ri, state):
                    """One projection group (q01 / q23 / kv) + its RoPE muls
                    + per-group rot_half swap DMA."""
                    t0 = it * TT
                    tsl = slice(t0 % S, t0 % S + TT)
                    if ri == 0:
                        state["m1"] = rp_pool.tile([128, 3, TT], bf16, tag="m1", name="m1")
                        state["z"] = rp_pool.tile([128, 3, TT], bf16, tag="z", name="z")
                        state["m2p"] = rp_pool.tile([128, 3, TT], bf16, tag="m2p", name="m2p")
                    m1_all, z_all, m2p_all = state["m1"], state["z"], state["m2p"]
                    wsrc, e0, e1, nrows = [
                        (wq_s, 0, 128, 128),
                        (wq_s, 128, 256, 128),
                        (wkv_s, 0, 2 * HD, 64),
                    ][ri]
                    ps = ps_mm.tile([128, TT], f32, tag="mm2k", name=f"pj{ri}")
                    for idb in range(NDB):
                        nc.tensor.matmul(
                            ps,
                            wsrc[:, idb, e0:e1],
                            htile[:, idb, :],
                            start=(idb == 0),
                            stop=(idb == NDB - 1),
                        )
                    if ri == 2:
                        # stage V to SBUF right away (ACT) so the V
                        # transposes don't wait on the DVE rope muls
                        v_sb = vs_pool.tile([128, TT], f32, tag="v_sb")
                        nc.scalar.copy(out=v_sb[64:128, :], in_=ps[64:128, :])
                        state["v_sb"] = v_sb
                    # RoPE input products; m2p (swapped z) comes via DMA
                    nc.vector.tensor_mul(
                        m1_all[:nrows, ri, :], ps[:nrows], cos_s[:nrows, tsl]
                    )
                    nc.vector.tensor_mul(
                        z_all[:nrows, ri, :], ps[:nrows], sinp_s[:nrows, tsl]
                    )
                    # partition swap of this group's z (32-row block pairs
                    # 0<->1, 2<->3) -- per group so consumers start early
                    nsw = 4 if nrows == 128 else 2
                    for c, lo in ((0, 32), (1, 0), (2, 96), (3, 64))[:nsw]:
                        nc.sync.dma_start(
                            out=m2p_all[c * 32 : c * 32 + 32, ri, :],
                            in_=z_all[lo : lo + 32, ri, :],
                        )

                def proj_tail(it, state):
                    """RoPE adds + V transpose for token tile it."""
                    t0 = it * TT
                    tsl = slice(t0, t0 + TT)
                    sl2 = it % 2
                    m1_all, m2p_all = state["m1"], state["m2p"]
                    # rope adds; k lands twice so odd q-heads can matmul
                    # from partition base 64 (tile_position row packing)
                    nc.vector.tensor_add(
                        kt_s[0:64, tsl], m1_all[0:64, 2, :], m2p_all[0:64, 2, :]
                    )
                    nc.vector.tensor_add(
                        kt_s[64:128, tsl], m1_all[0:64, 2, :], m2p_all[0:64, 2, :]
                    )
                    nc.vector.tensor_add(
                        qt_s[0][:, sl2, :], m1_all[:, 0, :], m2p_all[:, 0, :]
                    )
                    nc.vector.tensor_add(
                        qt_s[1][:, sl2, :], m1_all[:, 1, :], m2p_all[:, 1, :]
                    )
                    # V: [d, t] -> [t, d] via PE transpose (V was staged
                    # to SBUF right after the kv projection)
                    v_sb = state["v_sb"]
                    for c4 in range(TT // 128):
                        vt_ps = ps_mm.tile([128, HD], f32, tag="mm2k", name="vt")
                        nc.tensor.transpose(
                            vt_ps,
                            v_sb[64:128, c4 * 128 : (c4 + 1) * 128],
                            ident[64:128, 64:128],
                        )
                        nc.vector.tensor_copy(
                            out=va_s[:, it * 4 + c4, 0:HD], in_=vt_ps
                        )

                def attn_block(it, astate, rp, kb, nkb, own_o=False):
                    """One 128-key attention block of query tile it."""
                    b, iq = it // 4, it % 4
                    q0 = iq * QT
                    sl2 = it % 2
                    qtile = qt_s[rp]
                    if kb == 0:
                        if own_o:
                            # last tile's rp1 gets its own accumulator banks
                            # (from the score pool) so it never waits on
                            # rp0's evacuation
                            so = ps_s.tile([128, 2, QT], f32, tag="s",
                                           name="so1")
                            astate[f"o{rp}"] = [so[0:65, i, :] for i in range(2)]
                        else:
                            # 2 PSUM banks shared by both head-pairs: rp1's
                            # first A@V slot-waits on rp0's evacuation copies
                            astate[f"o{rp}"] = [
                                ps_o.tile(
                                    [65, QT], f32, tag=f"o{i}", name=f"o{i}", bufs=1
                                )
                                for i in range(2)
                            ]
                    o_ps = astate[f"o{rp}"]
                    ksl = slice(b * S + kb * KBLK, b * S + (kb + 1) * KBLK)
                    j = kb - q0 // KBLK
                    # query-column truncation: straddle block j only
                    # touches queries f >= 128*j
                    c0 = 128 * j if (causal and j > 0) else 0
                    s_ps = ps_s.tile([128, 2, QT], f32, tag="s")
                    pt = pt_pool.tile([128, 2, QT], bf16, tag="pt")
                    for h in range(2):
                        hb = h * 64
                        nc.tensor.matmul(
                            s_ps[:, h, c0:QT],
                            kt_s[hb : hb + 64, ksl],
                            qtile[hb : hb + 64, sl2, c0:QT],
                            start=True,
                            stop=True,
                        )
                    if causal:
                        nc.scalar.activation(
                            pt[:, :, c0:QT], s_ps[:, :, c0:QT], Exp, scale=0.125
                        )
                    else:
                        mk = pt_pool.tile([128, QT], f32, tag="mk")
                        sm = pt_pool.tile([128, 2, QT], f32, tag="sm")
                        nc.sync.dma_start(
                            out=mk,
                            in_=maskT[kb * KBLK : (kb + 1) * KBLK, q0 : q0 + QT],
                        )
                        for h in range(2):
                            nc.vector.scalar_tensor_tensor(
                                out=sm[:, h, :],
                                in0=s_ps[:, h, :],
                                scalar=0.125,
                                in1=mk,
                                op0=mybir.AluOpType.mult,
                                op1=mybir.AluOpType.add,
                            )
                        nc.scalar.activation(pt, sm, Exp, scale=1.0)
                    for h in range(2):
                        if causal and j >= 0:
                            # zero the sub-diagonal triangle in place on the
                            # (otherwise idle) gpsimd: keep where (f-c0) >= p.
                            # Only the first 128 query columns of a straddle
                            # block can be sub-diagonal (f-c0 >= 128 > any p)
                            nc.gpsimd.affine_select(
                                out=pt[:, h, c0 : c0 + KBLK],
                                in_=pt[:, h, c0 : c0 + KBLK],
                                compare_op=mybir.AluOpType.is_ge,
                                fill=0.0,
                                base=0,
                                channel_multiplier=-1,
                                pattern=[[1, KBLK]],
                            )
                        nc.tensor.matmul(
                            o_ps[h][:, c0:QT],
                            va_s[:, b * (S // 128) + kb, :],
                            pt[:, h, c0:QT],
                            start=(kb == 0),
                            stop=(kb == nkb - 1),
                        )

                def evac_rp(astate, rp, on_act=False):
                    """Evacuate the pair's A@V accumulators (frees the two
                    o PSUM banks for the next head pair).  on_act: use the
                    scalar engine (last tile -- DVE is busy with Wo evacs
                    and ACT's exp queue is exactly where this must not wait)."""
                    ou_all = astate["ou"]
                    o_ps = astate[f"o{rp}"]
                    for h in range(2):
                        if on_act:
                            nc.scalar.copy(
                                out=ou_all[:, rp * 2 + h, :], in_=o_ps[h]
                            )
                        else:
                            nc.vector.tensor_copy(
                                out=ou_all[:, rp * 2 + h, :], in_=o_ps[h]
                            )

                def normalize_rp(astate, rp):
                    """Per-head-pair softmax normalization (used for the
                    last tile).  Low-latency: SBUF gather -> 32-lane
                    reciprocal -> SBUF scatter -> PE-matmul broadcast
                    (no DRAM bounce)."""
                    on_t, ou_all = astate["on_t"], astate["ou"]
                    r32 = nm_pool.tile([32, 32], f32, tag="r32", name=f"r32{rp}", bufs=2)
                    nc.sync.dma_start(
                        out=r32, in_=ou_all[64:65, rp * 2 : rp * 2 + 2, :]
                    )
                    r32r = nm_pool.tile([32, 32], bf16, tag="r32r", name=f"r32r{rp}", bufs=2)
                    with nc.allow_low_precision(reason="bf16 recip broadcast"):
                        nc.vector.reciprocal(r32r, r32)
                    rec1 = nm_pool.tile([1, 2 * QT], bf16, tag="rec1", name=f"rec1{rp}", bufs=2)
                    nc.sync.dma_start(out=rec1, in_=r32r)
                    # both heads' broadcasts live in one mm2k bank: h0 on
                    # partitions 0:64, h1 on 64:128 (col tile_position 64)
                    bc_ps = ps_mm.tile([128, QT], f32, tag="mm2k", name=f"bc{rp}")
                    for h in range(2):
                        nc.tensor.matmul(
                            bc_ps[h * 64 : h * 64 + 64, :],
                            ones64,
                            rec1[:, h * QT : (h + 1) * QT],
                            start=True,
                            stop=True,
                        )
                        nc.vector.tensor_mul(
                            on_t[rp][h * 64 : h * 64 + 64, :],
                            ou_all[0:64, rp * 2 + h, :],
                            bc_ps[h * 64 : h * 64 + 64, :],
                        )

                def normalize_tail(astate):
                    """Batched softmax normalization for all 4 heads: the
                    denominator rows bounce through a [32, 64] layout so
                    reciprocal uses 32 lanes, and the partition broadcast is
                    a stride-0 DMA through a DRAM scratch (no engine time)."""
                    on_t, ou_all = astate["on_t"], astate["ou"]
                    r32 = nm_pool.tile([32, 64], f32, tag="r32", name="r32", bufs=2)
                    nc.sync.dma_start(out=r32, in_=ou_all[64:65, :, :])
                    r32r = nm_pool.tile([32, 64], f32, tag="r32r", name="r32r", bufs=2)
                    nc.vector.reciprocal(r32r, r32)
                    rd = dr_pool.tile([1, 4 * QT], f32, tag="rd", name="rd")
                    nc.sync.dma_start(out=rd, in_=r32r)
                    rec_b = nm_pool.tile([64, 4 * QT], f32, tag="rb", name="rb", bufs=2)
                    nc.sync.dma_start(
                        out=rec_b, in_=rd.partition_broadcast(64)[:, 0, :]
                    )
                    for rp in range(2):
                        for h in range(2):
                            hh = rp * 2 + h
                            nc.vector.tensor_mul(
                                on_t[rp][h * 64 : h * 64 + 64, :],
                                ou_all[0:64, hh, :],
                                rec_b[:, hh * QT : (hh + 1) * QT],
                            )

                def attn_begin(it):
                    b, iq = it // 4, it % 4
                    nkb = (iq * QT // KBLK + 4) if causal else (S // KBLK)
                    astate = {
                        "on_t": [
                            on_pool.tile(
                                [128, QT], bf16, tag=f"on{i}", name=f"on{i}"
                            )
                            for i in range(2)
                        ],
                        "ou": nm_pool.tile([65, 4, QT], f32, tag="ou", name="ou", bufs=2),
                        "nkb": nkb,
                    }
                    return astate

                def attn_out(it, on_t):
                    """Wo projection + coalesced bf16 output store.  During
                    exp-heavy steps (attention tile (it+1)%4 in {2,3}) the
                    PSUM evacuations go entirely to DVE, keeping ACT free
                    for exps."""
                    b, iq = it // 4, it % 4
                    q0 = iq * QT
                    qsl = slice(b * S + q0, b * S + q0 + QT)
                    heavy = (it % 4) in (1, 2)
                    out_acc = oa_pool.tile([128, D // 128, QT], bf16, tag="oacc")
                    for eb in range(D // 128):
                        wo_ps = ps_mm.tile([128, QT], f32, tag="mm2k", name="wo")
                        for db in range(2):
                            nc.tensor.matmul(
                                wo_ps,
                                wo_s[:, db, eb * 128 : (eb + 1) * 128],
                                on_t[db],
                                start=(db == 0),
                                stop=(db == 1),
                            )
                        if (not heavy) and eb % 2 == 1:
                            nc.scalar.copy(out=out_acc[:, eb, :], in_=wo_ps)
                        else:
                            nc.vector.tensor_copy(
                                out=out_acc[:, eb, :], in_=wo_ps
                            )
                    half = D // 256
                    nc.sync.dma_start(
                        out=outP_b[:, it, 0:half, :], in_=out_acc[:, 0:half, :]
                    )
                    nc.sync.dma_start(
                        out=outP_b[:, it, half:, :], in_=out_acc[:, half:, :]
                    )

                def attn_out_half(it, on_t, rp):
                    """One head-pair's Wo partial for the last tile.  rp0
                    goes to the usual outP slot (overlapping rp1's
                    attention), rp1 to outP2 (host adds).  The rp1 half is
                    the kernel tail: nothing else runs, so it gets extra
                    PSUM accumulator slots from the (now idle) score banks
                    and stores in quarters so the final DMA lands early."""
                    out_acc = oa_pool.tile(
                        [128, D // 128, QT], bf16, tag="oacc", name=f"oah{rp}"
                    )
                    sx = None
                    for eb in range(D // 128):
                        if rp == 1 and eb % 4 == 2:
                            # two extra banks per s-tag tile
                            sx = ps_s.tile([128, 2, QT], f32, tag="s",
                                           name=f"wx{eb}")
                        if rp == 1 and eb % 4 >= 2:
                            wo_ps = sx[:, eb % 4 - 2, :]
                        else:
                            wo_ps = ps_mm.tile([128, QT], f32, tag="mm2k",
                                               name="woh")
                        nc.tensor.matmul(
                            wo_ps,
                            wo_s[:, rp, eb * 128 : (eb + 1) * 128],
                            on_t[rp],
                            start=True,
                            stop=True,
                        )
                        if eb % 2 == 1:
                            nc.scalar.copy(out=out_acc[:, eb, :], in_=wo_ps)
                        else:
                            nc.vector.tensor_copy(
                                out=out_acc[:, eb, :], in_=wo_ps
                            )
                        if rp == 1 and eb % 4 == 3:
                            nc.sync.dma_start(
                                out=outP2_b[:, eb - 3 : eb + 1, :],
                                in_=out_acc[:, eb - 3 : eb + 1, :],
                            )
                    if rp == 0:
                        half = D // 256
                        nc.sync.dma_start(
                            out=outP_b[:, it, 0:half, :], in_=out_acc[:, 0:half, :]
                        )
                        nc.sync.dma_start(
                            out=outP_b[:, it, half:, :], in_=out_acc[:, half:, :]
                        )

                # ---------------- the software-pipelined merged loop:
                # attention for tile s-1, projection for tile s, Wo for tile
                # s-2.  The 2-step Wo skew means the Wo matmuls' inputs are
                # always long-ready (they fill PE gaps, and their PSUM
                # evacuation copies never head-of-line-block the exps), and
                # the normalize chain of s-1 has a full step to complete.
                htiles = {0: prefetch(0, split=True)}
                # the rest of the constants, behind ht0 in the scalar FIFO;
                # ht1 rides the same FIFO so it cannot be hoisted ahead of
                # the step-0 critical loads
                nc.scalar.dma_start(out=wq_s[:, 8:16, :], in_=wqP_b[:, 8:16, :])
                nc.scalar.dma_start(out=wkv_s, in_=wkvP_b)
                nc.scalar.dma_start(out=cos_s[64:128, :], in_=cos_s[0:64, :])
                nc.scalar.dma_start(out=sinp_s[64:128, :], in_=sinp_s[0:64, :])
                htiles[1] = prefetch(1, on_scalar=True)
                nc.scalar.dma_start(out=wo_s, in_=woP_b)
                on_hist = {}
                for step in range(NT + 2):
                    if step == NT and (step - 2) in on_hist:
                        # tail step: emit the Wo of tile s-2 before the last
                        # attention so its matmuls aren't queued behind it
                        attn_out(step - 2, on_hist.pop(step - 2))
                    if 1 <= step <= NT:
                        it_a = step - 1
                        astate = attn_begin(it_a)
                        nkb = astate["nkb"]
                        last = it_a == NT - 1
                        for rp in range(2):
                            for kb in range(nkb):
                                attn_block(it_a, astate, rp, kb, nkb)
                            evac_rp(astate, rp, on_act=last)
                            if last:
                                normalize_rp(astate, rp)
                                attn_out_half(it_a, astate["on_t"], rp)
                        if not last:
                            normalize_tail(astate)
                            on_hist[it_a] = astate["on_t"]
                    if step <= NT - 1:
                        pstate = {}
                        htile = htiles.pop(step)
                        for ri in range(3):
                            proj_chunk(step, htile, ri, pstate)
                        proj_tail(step, pstate)
                    # prefetch after the step body so early hT transfers
                    # don't steal HBM bandwidth from the weight/RoPE loads
                    if step + 1 <= NT - 1 and (step + 1) not in htiles:
                        htiles[step + 1] = prefetch(step + 1)
                    if step >= 1 and step + 2 <= NT - 1:
                        htiles[step + 2] = prefetch(step + 2)
                    if step >= 2 and (step - 2) in on_hist:
                        attn_out(step - 2, on_hist.pop(step - 2))

    nc.compile()
    return nc


def _host_inputs(inputs, causal):
    """Shard + transpose the full inputs into 8 per-core input maps."""
    h = np.asarray(inputs["hidden_states"], np.float32)
    cos = np.asarray(inputs["position_cos"], np.float32)
    sin = np.asarray(inputs["position_sin"], np.float32)
    Wq = np.asarray(inputs["Wq"], np.float32)
    Wk = np.asarray(inputs["Wk"], np.float32)
    Wv = np.asarray(inputs["Wv"], np.float32)
    Wo = np.asarray(inputs["Wo"], np.float32)
    mask = np.asarray(inputs["attention_mask"], np.float32)[0, 0]

    # hT [D, T] -> [128, NT, NDB, TT]: partition p, tile it line contiguous
    hT = h.reshape(T, D).T.astype(BF16)                  # [D, T]
    hTP = np.ascontiguousarray(
        hT.reshape(NDB, 128, NT, TT).transpose(1, 2, 0, 3).reshape(128, -1)
    )

    cosT = cos.T                                      # [64, S]
    sinT = sin.T
    cosD = np.ascontiguousarray(cosT.astype(BF16))
    s_signed = np.vstack([-sinT[0:32], sinT[32:64]])  # rot_half sign baked in
    # pre-swap so that z[p] = x[p]*sinp[p]; m2[p] = z[swap(p)] equals
    # rot_half(x)[p] * sin_signed[p]  (swap = 32-row block pairs 0<->1;
    # rows 64:128 and batch 1 are expanded on-device)
    swap_idx = np.concatenate([np.arange(32, 64), np.arange(0, 32)])
    sinD = np.ascontiguousarray(s_signed[swap_idx].astype(BF16))

    maskT = np.ascontiguousarray(mask.T).astype(np.float32)

    def pack(w):
        # [R, C] with R = n*128 -> [128, n*C] so partition lines are
        # contiguous in DRAM (one fat DMA descriptor per partition)
        r, c = w.shape
        n = r // 128
        return np.ascontiguousarray(
            w.reshape(n, 128, c).transpose(1, 0, 2).reshape(128, n * c)
        ).astype(BF16)

    in_maps = []
    for g in range(8):
        wqT = Wq[g * EQ : (g + 1) * EQ].T              # [D, EQ]
        wkvT = np.concatenate(
            [Wk[g * HD : (g + 1) * HD].T, Wv[g * HD : (g + 1) * HD].T], axis=1
        )                                               # [D, 128]
        woT = Wo[:, g * EQ : (g + 1) * EQ].T            # [EQ, D]
        in_maps.append(
            {
                "hTP": hTP,
                "wqP": pack(wqT),
                "wkvP": pack(wkvT),
                "woP": pack(woT),
                "cosD": cosD,
                "sinD": sinD,
                "maskT": maskT,
            }
        )
    return in_maps


def _is_causal(mask):
    m = np.asarray(mask, np.float32)[0, 0]
    tri = np.tril(np.ones((S, S), bool))
    return bool(np.all(m[tri] == 0.0) and np.all(m[~tri] <= -1e8))


def _assemble(results):
    # outP [128, NT, NDB, TT] -> [D, T]; outP2 [128, NDB, QT] adds into
    # the last query tile's columns
    acc = np.zeros((128, NT, NDB, TT), np.float32)
    for r in results:
        acc += r["outP"].astype(np.float32).reshape(128, NT, NDB, TT)
        acc[:, NT - 1] += r["outP2"].astype(np.float32).reshape(128, NDB, QT)
    out = acc.transpose(2, 0, 1, 3).reshape(D, T)   # [NDB*128=D, NT*TT=T]
    return np.ascontiguousarray(out.reshape(D, B, S).transpose(1, 2, 0))


def kernel(**inputs) -> np.ndarray:
    from concourse.bass_utils import run_bass_kernel_spmd

    causal = _is_causal(inputs["attention_mask"])
    key = ("prog", causal)
    if key not in _CACHE:
        _CACHE[key] = _build_program(causal)
    nc = _CACHE[key]

    in_maps = _host_inputs(inputs, causal)
    res = run_bass_kernel_spmd(nc, in_maps, core_ids=list(range(8)))
    return _assemble(res.results)


# revision 43
# speedup vs baseline: 1.0058x; 1.0058x over previous
"""Trainium2 Bass kernel for GQA multi-head attention (nn_MultiHeadAttention).

Reference computation (fp32):
    q = h @ Wq^T -> RoPE ; k = h @ Wk^T -> RoPE ; v = h @ Wv^T
    scores = q k^T / sqrt(64) + causal_mask ; w = softmax(scores)
    out = (w v) @ Wo^T

Shapes: h [2,2048,2048], Wq [2048,2048], Wk/Wv [512,2048], Wo [2048,2048],
32 q heads / 8 kv heads (GQA group=4), head_dim 64.

Sharding: tensor-parallel over the 8 kv-head groups, one group per core.
Core g owns q heads [4g,4g+4), kv head g, Wo columns [256g, 256(g+1)).
Each core computes a full-token partial of the output projection; the host
sums the 8 partials (the Wo contraction splits over head blocks).

Structure: a single software-pipelined loop over the 8 token tiles of 512;
at step s it emits [hT prefetch for s+1] [attention for query tile s-1]
[QKV projection + RoPE for tile s] [Wo projection + output store for tile
s-2].  All weights are packed host-side so every DMA moves whole-partition
contiguous lines (128 fat descriptors instead of 2048 thin ones), and the
weight loads ride the scalar HWDGE ring while hT tiles ride the sync ring,
so the first matmul starts ~8us in instead of ~37us.  RoPE scratch is bf16
double-buffered; the rot_half partition swap runs per projection group so
the q/k tiles are ready group-by-group.  The Wo PSUM evacuations move to
DVE during exp-heavy steps (ACT is the attention-phase bottleneck).  The
last query tile's Wo projection is split into its two head-pair halves
written to separate DRAM partials (host adds them) so half of it overlaps
the final exp storm.
"""

import sys

for _p in ("/opt/trn_rl_repo",):
    if _p not in sys.path:
        sys.path.insert(0, _p)

import numpy as np
import ml_dtypes

D = 2048          # model dim
HD = 64           # head dim
S = 2048          # sequence
B = 2             # batch
T = B * S         # total tokens
EQ = 256          # q-projection rows per core (4 heads x 64)
TT = 512          # token tile (both projection and query tile)
NT = T // TT      # 8 merged steps
NDB = D // 128    # contraction blocks for projections
QT = 512          # query tile for attention
KBLK = 128        # key block for attention
BF16 = ml_dtypes.bfloat16

_CACHE = {}


def _build_program(causal: bool):
    """Build the single-core Bass/Tile program (identical across cores)."""
    import concourse.bass as bass
    import concourse.mybir as mybir
    import concourse.tile as tile
    from concourse import bacc
    from concourse.masks import make_identity

    f32 = mybir.dt.float32
    bf16 = mybir.dt.bfloat16

    nc = bacc.Bacc("TRN2", target_bir_lowering=False, debug=False)

    # hT packed host-side as [128, NT, 16, TT] so each (partition, tile)
    # line is one contiguous 16 KiB DMA descriptor
    hTP = nc.dram_tensor("hTP", [128, NT * NDB * TT], bf16, kind="ExternalInput").ap()
    # weights packed so partition lines are contiguous in DRAM
    wqP = nc.dram_tensor("wqP", [128, NDB * EQ], bf16, kind="ExternalInput").ap()
    wkvP = nc.dram_tensor("wkvP", [128, NDB * 2 * HD], bf16, kind="ExternalInput").ap()
    woP = nc.dram_tensor("woP", [128, 2 * D], bf16, kind="ExternalInput").ap()
    # deduplicated RoPE tables: [64, S] (the [128, T] SBUF tiles are built
    # by SBUF->SBUF copies -- rows 64:128 and the second batch repeat)
    cosD = nc.dram_tensor("cosD", [64, S], bf16, kind="ExternalInput").ap()
    # sin with rot_half sign AND partition swap pre-applied (see _host_inputs)
    sinD = nc.dram_tensor("sinD", [64, S], bf16, kind="ExternalInput").ap()
    # mask^T tiles, only used when causal=False
    maskT = nc.dram_tensor("maskT", [S, S], f32, kind="ExternalInput").ap()
    # outputs packed the same way: [128, NT, 16, TT] / [128, 16, QT]
    outP = nc.dram_tensor("outP", [128, NT * NDB * TT], bf16, kind="ExternalOutput").ap()
    # rp1 partial of the last query tile's Wo projection (host adds)
    outP2 = nc.dram_tensor("outP2", [128, NDB * QT], bf16, kind="ExternalOutput").ap()

    hT_b3 = hTP.rearrange("p (i n t) -> p i n t", i=NT, n=NDB)  # [128, 8, 16, TT]
    wqP_b = wqP.rearrange("p (n e) -> p n e", n=NDB)
    wkvP_b = wkvP.rearrange("p (n e) -> p n e", n=NDB)
    woP_b = woP.rearrange("p (n e) -> p n e", n=2)
    outP_b = outP.rearrange("p (i n t) -> p i n t", i=NT, n=NDB)  # [128, 8, 16, TT]
    outP2_b = outP2.rearrange("p (n t) -> p n t", n=NDB)          # [128, 16, QT]

    Exp = mybir.ActivationFunctionType.Exp
    PSUM = bass.MemorySpace.PSUM

    with tile.TileContext(nc) as tc:
        import contextlib

        with contextlib.ExitStack() as stack:
            const = stack.enter_context(tc.tile_pool(name="const", bufs=1))

            wq_s = const.tile([128, NDB, EQ], bf16)
            wkv_s = const.tile([128, NDB, 2 * HD], bf16)
            wo_s = const.tile([128, 2, D], bf16)
            # RoPE tables indexed mod S (both batches share positions)
            cos_s = const.tile([128, S], bf16)
            sinp_s = const.tile([128, S], bf16)
            # q tiles: 2-slot ring per head pair (attn reads slot (it)%2
            # while proj writes slot (it+1)%2)
            qt_s = [
                const.tile([128, 2, TT], bf16, tag=f"qt{i}", name=f"qt{i}")
                for i in range(2)
            ]
            kt_s = const.tile([128, T], bf16)
            va_s = const.tile([128, T // 128, HD + 1], bf16)
            tri_s = const.tile([128, 4, QT], bf16)
            ident = const.tile([128, 128], f32)
            ones64 = const.tile([1, 64], bf16)

            # weight loads on the scalar HWDGE ring (parallel with hT tiles
            # on the sync ring).  wq split in halves so the first projection
            # matmuls only wait for the first half.
            # strict need-order on ONE ring (single-ring FIFO = exact
            # bandwidth priority; ht tile 0 is emitted right here too, see
            # the loop below): wq first half, RoPE tables, ht0, the rest
            nc.scalar.dma_start(out=wq_s[:, 0:8, :], in_=wqP_b[:, 0:8, :])
            nc.scalar.dma_start(out=cos_s[0:64, :], in_=cosD)
            nc.scalar.dma_start(out=sinp_s[0:64, :], in_=sinD)
            nc.scalar.dma_start(out=wq_s[:, 8:16, :], in_=wqP_b[:, 8:16, :])
            nc.scalar.dma_start(out=wkv_s, in_=wkvP_b)

            make_identity(nc, ident)
            # ones column of the augmented V
            nc.gpsimd.memset(va_s[:, :, HD : HD + 1], 1.0)
            nc.gpsimd.memset(ones64, 1.0)
            # multiplicative causal masks for the 4 straddle offsets:
            # tri_s[p, j, f] = 1.0 where f >= p + 128*j else 0.0
            for j in range(4):
                nc.gpsimd.memset(tri_s[:, j, :], 1.0)
                nc.gpsimd.affine_select(
                    out=tri_s[:, j, :],
                    in_=tri_s[:, j, :],
                    compare_op=mybir.AluOpType.is_ge,
                    fill=0.0,
                    base=-128 * j,
                    channel_multiplier=-1,
                    pattern=[[1, QT]],
                )

            # ---------------- pools for the merged pipeline
            with contextlib.ExitStack() as pp:
                ht_pool = pp.enter_context(tc.tile_pool(name="ht", bufs=3))
                # m1 / z / swapped-z rope scratch (bf16, double-buffered)
                rp_pool = pp.enter_context(tc.tile_pool(name="rp", bufs=2))
                vs_pool = pp.enter_context(tc.tile_pool(name="vs", bufs=2))
                # shared-PSUM pool: proj accumulators, V transposes, Wo tiles
                ps_mm = pp.enter_context(
                    tc.tile_pool(name="ps_mm", bufs=2, space=PSUM)
                )
                ps_s = pp.enter_context(
                    tc.tile_pool(name="ps_s", bufs=2, space=PSUM)
                )
                ps_o = pp.enter_context(
                    tc.tile_pool(name="ps_o", bufs=1, space=PSUM)
                )
                pt_pool = pp.enter_context(tc.tile_pool(name="pt", bufs=6))
                on_pool = pp.enter_context(tc.tile_pool(name="on", bufs=2))
                nm_pool = pp.enter_context(tc.tile_pool(name="nm", bufs=1))
                oa_pool = pp.enter_context(tc.tile_pool(name="oa", bufs=1))
                dr_pool = pp.enter_context(
                    tc.tile_pool(name="dr", bufs=2, space="DRAM")
                )

                def prefetch(it, split=False, on_scalar=False):
                    htile = ht_pool.tile([128, NDB, TT], bf16, tag="ht",
                                         name=f"ht{it}")
                    eng = nc.scalar if (split or on_scalar) else nc.sync
                    if split:
                        eng.dma_start(
                            out=htile[:, 0:8, :], in_=hT_b3[:, it, 0:8, :]
                        )
                        eng.dma_start(
                            out=htile[:, 8:16, :], in_=hT_b3[:, it, 8:16, :]
                        )
                    else:
                        eng.dma_start(out=htile, in_=hT_b3[:, it, :, :])
                    return htile

                def proj_chunk(it, htile, ri, state):
                    """One projection group (q01 / q23 / kv) + its RoPE muls
                    + per-group rot_half swap DMA."""
                    t0 = it * TT
                    tsl = slice(t0 % S, t0 % S + TT)
                    if ri == 0:
                        state["m1"] = rp_pool.tile([128, 3, TT], bf16, tag="m1", name="m1")
                        state["z"] = rp_pool.tile([128, 3, TT], bf16, tag="z", name="z")
                        state["m2p"] = rp_pool.tile([128, 3, TT], bf16, tag="m2p", name="m2p")
                    m1_all, z_all, m2p_all = state["m1"], state["z"], state["m2p"]
                    wsrc, e0, e1, nrows = [
                        (wq_s, 0, 128, 128),
                        (wq_s, 128, 256, 128),
                        (wkv_s, 0, 2 * HD, 64),
                    ][ri]
                    ps = ps_mm.tile([128, TT], f32, tag="mm2k", name=f"pj{ri}")
                    for idb in range(NDB):
                        nc.tensor.matmul(
                            ps,
                            wsrc[:, idb, e0:e1],
                            htile[:, idb, :],
                            start=(idb == 0),
                            stop=(idb == NDB - 1),
                        )
                    if ri == 2:
                        # stage V to SBUF right away (ACT) so the V
                        # transposes don't wait on the DVE rope muls
                        v_sb = vs_pool.tile([128, TT], f32, tag="v_sb")
                        nc.scalar.copy(out=v_sb[64:128, :], in_=ps[64:128, :])
                        state["v_sb"] = v_sb
                    # RoPE input products; m2p (swapped z) comes via DMA
                    nc.vector.tensor_mul(
                        m1_all[:nrows, ri, :], ps[:nrows], cos_s[:nrows, tsl]
                    )
                    nc.vector.tensor_mul(
                        z_all[:nrows, ri, :], ps[:nrows], sinp_s[:nrows, tsl]
                    )
                    # partition swap of this group's z (32-row block pairs
                    # 0<->1, 2<->3) -- per group so consumers start early
                    nsw = 4 if nrows == 128 else 2
                    for c, lo in ((0, 32), (1, 0), (2, 96), (3, 64))[:nsw]:
                        nc.sync.dma_start(
                            out=m2p_all[c * 32 : c * 32 + 32, ri, :],
                            in_=z_all[lo : lo + 32, ri, :],
                        )

                def proj_tail(it, state):
                    """RoPE adds + V transpose for token tile it."""
                    t0 = it * TT
                    tsl = slice(t0, t0 + TT)
                    sl2 = it % 2
                    m1_all, m2p_all = state["m1"], state["m2p"]
                    # rope adds; k lands twice so odd q-heads can matmul
                    # from partition base 64 (tile_position row packing)
                    nc.vector.tensor_add(
                        kt_s[0:64, tsl], m1_all[0:64, 2, :], m2p_all[0:64, 2, :]
                    )
                    nc.vector.tensor_add(
                        kt_s[64:128, tsl], m1_all[0:64, 2, :], m2p_all[0:64, 2, :]
                    )
                    nc.vector.tensor_add(
                        qt_s[0][:, sl2, :], m1_all[:, 0, :], m2p_all[:, 0, :]
                    )
                    nc.vector.tensor_add(
                        qt_s[1][:, sl2, :], m1_all[:, 1, :], m2p_all[:, 1, :]
                    )
                    # V: [d, t] -> [t, d] via PE transpose (V was staged
                    # to SBUF right after the kv projection)
                    v_sb = state["v_sb"]
                    for c4 in range(TT // 128):
                        vt_ps = ps_mm.tile([128, HD], f32, tag="mm2k", name="vt")
                        nc.tensor.transpose(
                            vt_ps,
                            v_sb[64:128, c4 * 128 : (c4 + 1) * 128],
                            ident[64:128, 64:128],
                        )
                        nc.vector.tensor_copy(
                            out=va_s[:, it * 4 + c4, 0:HD], in_=vt_ps
                        )

                def attn_block(it, astate, rp, kb, nkb, own_o=False):
                    """One 128-key attention block of query tile it."""
                    b, iq = it // 4, it % 4
                    q0 = iq * QT
                    sl2 = it % 2
                    qtile = qt_s[rp]
                    if kb == 0:
                        if own_o:
                            # last tile's rp1 gets its own accumulator banks
                            # (from the score pool) so it never waits on
                            # rp0's evacuation
                            so = ps_s.tile([128, 2, QT], f32, tag="s",
                                           name="so1")
                            astate[f"o{rp}"] = [so[0:65, i, :] for i in range(2)]
                        else:
                            # 2 PSUM banks shared by both head-pairs: rp1's
                            # first A@V slot-waits on rp0's evacuation copies
                            astate[f"o{rp}"] = [
                                ps_o.tile(
                                    [65, QT], f32, tag=f"o{i}", name=f"o{i}", bufs=1
                                )
                                for i in range(2)
                            ]
                    o_ps = astate[f"o{rp}"]
                    ksl = slice(b * S + kb * KBLK, b * S + (kb + 1) * KBLK)
                    j = kb - q0 // KBLK
                    # query-column truncation: straddle block j only
                    # touches queries f >= 128*j
                    c0 = 128 * j if (causal and j > 0) else 0
                    s_ps = ps_s.tile([128, 2, QT], f32, tag="s")
                    pt = pt_pool.tile([128, 2, QT], bf16, tag="pt")
                    for h in range(2):
                        hb = h * 64
                        nc.tensor.matmul(
                            s_ps[:, h, c0:QT],
                            kt_s[hb : hb + 64, ksl],
                            qtile[hb : hb + 64, sl2, c0:QT],
                            start=True,
                            stop=True,
                        )
                    if causal:
                        nc.scalar.activation(
                            pt[:, :, c0:QT], s_ps[:, :, c0:QT], Exp, scale=0.125
                        )
                    else:
                        mk = pt_pool.tile([128, QT], f32, tag="mk")
                        sm = pt_pool.tile([128, 2, QT], f32, tag="sm")
                        nc.sync.dma_start(
                            out=mk,
                            in_=maskT[kb * KBLK : (kb + 1) * KBLK, q0 : q0 + QT],
                        )
                        for h in range(2):
                            nc.vector.scalar_tensor_tensor(
                                out=sm[:, h, :],
                                in0=s_ps[:, h, :],
                                scalar=0.125,
                                in1=mk,
                                op0=mybir.AluOpType.mult,
                                op1=mybir.AluOpType.add,
                            )
                        nc.scalar.activation(pt, sm, Exp, scale=1.0)
                    for h in range(2):
                        if causal and j >= 0:
                            # zero the sub-diagonal triangle in place on the
                            # (otherwise idle) gpsimd: keep where (f-c0) >= p.
                            # Only the first 128 query columns of a straddle
                            # block can be sub-diagonal (f-c0 >= 128 > any p)
                            nc.gpsimd.affine_select(
                                out=pt[:, h, c0 : c0 + KBLK],
                                in_=pt[:, h, c0 : c0 + KBLK],
                                compare_op=mybir.AluOpType.is_ge,
                                fill=0.0,
                                base=0,
                                channel_multiplier=-1,
                                pattern=[[1, KBLK]],
                            )
                        nc.tensor.matmul(
                            o_ps[h][:, c0:QT],
                            va_s[:, b * (S // 128) + kb, :],
                            pt[:, h, c0:QT],
                            start=(kb == 0),
                            stop=(kb == nkb - 1),
                        )

                def evac_rp(astate, rp, on_act=False):
                    """Evacuate the pair's A@V accumulators (frees the two
                    o PSUM banks for the next head pair).  on_act: use the
                    scalar engine (last tile -- DVE is busy with Wo evacs
                    and ACT's exp queue is exactly where this must not wait)."""
                    ou_all = astate["ou"]
                    o_ps = astate[f"o{rp}"]
                    for h in range(2):
                        if on_act:
                            nc.scalar.copy(
                                out=ou_all[:, rp * 2 + h, :], in_=o_ps[h]
                            )
                        else:
                            nc.vector.tensor_copy(
                                out=ou_all[:, rp * 2 + h, :], in_=o_ps[h]
                            )

                def normalize_rp(astate, rp):
                    """Per-head-pair softmax normalization (used for the
                    last tile).  Low-latency: SBUF gather -> 32-lane
                    reciprocal -> SBUF scatter -> PE-matmul broadcast
                    (no DRAM bounce)."""
                    on_t, ou_all = astate["on_t"], astate["ou"]
                    r32 = nm_pool.tile([32, 32], f32, tag="r32", name=f"r32{rp}", bufs=2)
                    nc.sync.dma_start(
                        out=r32, in_=ou_all[64:65, rp * 2 : rp * 2 + 2, :]
                    )
                    r32r = nm_pool.tile([32, 32], bf16, tag="r32r", name=f"r32r{rp}", bufs=2)
                    with nc.allow_low_precision(reason="bf16 recip broadcast"):
                        nc.vector.reciprocal(r32r, r32)
                    rec1 = nm_pool.tile([1, 2 * QT], bf16, tag="rec1", name=f"rec1{rp}", bufs=2)
                    nc.sync.dma_start(out=rec1, in_=r32r)
                    # both heads' broadcasts live in one mm2k bank: h0 on
                    # partitions 0:64, h1 on 64:128 (col tile_position 64)
                    bc_ps = ps_mm.tile([128, QT], f32, tag="mm2k", name=f"bc{rp}")
                    for h in range(2):
                        nc.tensor.matmul(
                            bc_ps[h * 64 : h * 64 + 64, :],
                            ones64,
                            rec1[:, h * QT : (h + 1) * QT],
                            start=True,
                            stop=True,
                        )
                        nc.vector.tensor_mul(
                            on_t[rp][h * 64 : h * 64 + 64, :],
                            ou_all[0:64, rp * 2 + h, :],
                            bc_ps[h * 64 : h * 64 + 64, :],
                        )

                def normalize_tail(astate):
                    """Batched softmax normalization for all 4 heads: the
                    denominator rows bounce through a [32, 64] layout so
                    reciprocal uses 32 lanes, and the partition broadcast is
                    a stride-0 DMA through a DRAM scratch (no engine time)."""
                    on_t, ou_all = astate["on_t"], astate["ou"]
                    r32 = nm_pool.tile([32, 64], f32, tag="r32", name="r32", bufs=2)
                    nc.sync.dma_start(out=r32, in_=ou_all[64:65, :, :])
                    r32r = nm_pool.tile([32, 64], f32, tag="r32r", name="r32r", bufs=2)
                    nc.vector.reciprocal(r32r, r32)
                    rd = dr_pool.tile([1, 4 * QT], f32, tag="rd", name="rd")
                    nc.sync.dma_start(out=rd, in_=r32r)
                    rec_b = nm_pool.tile([64, 4 * QT], f32, tag="rb", name="rb", bufs=2)
                    nc.sync.dma_start(
                        out=rec_b, in_=rd.partition_broadcast(64)[:, 0, :]
                    )
                    for rp in range(2):
                        for h in range(2):
                            hh = rp * 2 + h
                            nc.vector.tensor_mul(
                                on_t[rp][h * 64 : h * 64 + 64, :],
                                ou_all[0:64, hh, :],
                                rec_b[:, hh * QT : (hh + 1) * QT],
                            )

                def attn_begin(it):
                    b, iq = it // 4, it % 4
                    nkb = (iq * QT // KBLK + 4) if causal else (S // KBLK)
                    astate = {
                        "on_t": [
                            on_pool.tile(
                                [128, QT], bf16, tag=f"on{i}", name=f"on{i}"
                            )
                            for i in range(2)
                        ],
                        "ou": nm_pool.tile([65, 4, QT], f32, tag="ou", name="ou", bufs=2),
                        "nkb": nkb,
                    }
                    return astate

                def attn_out(it, on_t):
                    """Wo projection + coalesced bf16 output store.  During
                    exp-heavy steps (attention tile (it+1)%4 in {2,3}) the
                    PSUM evacuations go entirely to DVE, keeping ACT free
                    for exps."""
                    b, iq = it // 4, it % 4
                    q0 = iq * QT
                    qsl = slice(b * S + q0, b * S + q0 + QT)
                    heavy = (it % 4) in (1, 2)
                    out_acc = oa_pool.tile([128, D // 128, QT], bf16, tag="oacc")
                    for eb in range(D // 128):
                        wo_ps = ps_mm.tile([128, QT], f32, tag="mm2k", name="wo")
                        for db in range(2):
                            nc.tensor.matmul(
                                wo_ps,
                                wo_s[:, db, eb * 128 : (eb + 1) * 128],
                                on_t[db],
                                start=(db == 0),
                                stop=(db == 1),
                            )
                        if (not heavy) and eb % 2 == 1:
                            nc.scalar.copy(out=out_acc[:, eb, :], in_=wo_ps)
                        else:
                            nc.vector.tensor_copy(
                                out=out_acc[:, eb, :], in_=wo_ps
                            )
                    half = D // 256
                    nc.sync.dma_start(
                        out=outP_b[:, it, 0:half, :], in_=out_acc[:, 0:half, :]
                    )
                    nc.sync.dma_start(
                        out=outP_b[:, it, half:, :], in_=out_acc[:, half:, :]
                    )

                def attn_out_half(it, on_t, rp):
                    """One head-pair's Wo partial for the last tile.  rp0
                    goes to the usual outP slot (overlapping rp1's
                    attention), rp1 to outP2 (host adds).  The rp1 half is
                    the kernel tail: nothing else runs, so it gets extra
                    PSUM accumulator slots from the (now idle) score banks
                    and stores in quarters so the final DMA lands early."""
                    out_acc = oa_pool.tile(
                        [128, D // 128, QT], bf16, tag="oacc", name=f"oah{rp}"
                    )
                    sx = None
                    for eb in range(D // 128):
                        if rp == 1 and eb % 4 == 2:
                            # two extra banks per s-tag tile
                            sx = ps_s.tile([128, 2, QT], f32, tag="s",
                                           name=f"wx{eb}")
                        if rp == 1 and eb % 4 >= 2:
                            wo_ps = sx[:, eb % 4 - 2, :]
                        else:
                            wo_ps = ps_mm.tile([128, QT], f32, tag="mm2k",
                                               name="woh")
                        nc.tensor.matmul(
                            wo_ps,
                            wo_s[:, rp, eb * 128 : (eb + 1) * 128],
                            on_t[rp],
                            start=True,
                            stop=True,
                        )
                        if eb % 2 == 1:
                            nc.scalar.copy(out=out_acc[:, eb, :], in_=wo_ps)
                        else:
                            nc.vector.tensor_copy(
                                out=out_acc[:, eb, :], in_=wo_ps
                            )
                        if rp == 1 and eb % 4 == 3:
                            nc.sync.dma_start(
                                out=outP2_b[:, eb - 3 : eb + 1, :],
                                in_=out_acc[:, eb - 3 : eb + 1, :],
                            )
                    if rp == 0:
                        half = D // 256
                        nc.sync.dma_start(
                            out=outP_b[:, it, 0:half, :], in_=out_acc[:, 0:half, :]
                        )
                        nc.sync.dma_start(
                            out=outP_b[:, it, half:, :], in_=out_acc[:, half:, :]
                        )

                # ---------------- the software-pipelined merged loop:
                # attention for tile s-1, projection for tile s, Wo for tile
                # s-2.  The 2-step Wo skew means the Wo matmuls' inputs are
                # always long-ready (they fill PE gaps, and their PSUM
                # evacuation copies never head-of-line-block the exps), and
                # the normalize chain of s-1 has a full step to complete.
                htiles = {0: prefetch(0, split=True)}
                # behind ht0 in the scalar FIFO; ht1 rides the same FIFO so
                # it cannot be hoisted ahead of the step-0 critical loads
                nc.scalar.dma_start(out=cos_s[64:128, :], in_=cos_s[0:64, :])
                nc.scalar.dma_start(out=sinp_s[64:128, :], in_=sinp_s[0:64, :])
                htiles[1] = prefetch(1, on_scalar=True)
                nc.scalar.dma_start(out=wo_s, in_=woP_b)
                on_hist = {}
                for step in range(NT + 2):
                    if step == NT and (step - 2) in on_hist:
                        # tail step: emit the Wo of tile s-2 before the last
                        # attention so its matmuls aren't queued behind it
                        attn_out(step - 2, on_hist.pop(step - 2))
                    if 1 <= step <= NT:
                        it_a = step - 1
                        astate = attn_begin(it_a)
                        nkb = astate["nkb"]
                        last = it_a == NT - 1
                        for rp in range(2):
                            for kb in range(nkb):
                                attn_block(it_a, astate, rp, kb, nkb)
                            evac_rp(astate, rp, on_act=last)
                            if last:
                                normalize_rp(astate, rp)
                                attn_out_half(it_a, astate["on_t"], rp)
                        if not last:
                            normalize_tail(astate)
                            on_hist[it_a] = astate["on_t"]
                    if step <= NT - 1:
                        pstate = {}
                        htile = htiles.pop(step)
                        for ri in range(3):
                            proj_chunk(step, htile, ri, pstate)
                        proj_tail(step, pstate)
                    # prefetch after the step body so early hT transfers
                    # don't steal HBM bandwidth from the weight/RoPE loads
                    if step + 1 <= NT - 1 and (step + 1) not in htiles:
                        htiles[step + 1] = prefetch(step + 1)
                    if step >= 1 and step + 2 <= NT - 1:
                        htiles[step + 2] = prefetch(step + 2)
                    if step >= 2 and (step - 2) in on_hist:
                        attn_out(step - 2, on_hist.pop(step - 2))

    nc.compile()
    return nc


def _host_inputs(inputs, causal):
    """Shard + transpose the full inputs into 8 per-core input maps."""
    h = np.asarray(inputs["hidden_states"], np.float32)
    cos = np.asarray(inputs["position_cos"], np.float32)
    sin = np.asarray(inputs["position_sin"], np.float32)
    Wq = np.asarray(inputs["Wq"], np.float32)
    Wk = np.asarray(inputs["Wk"], np.float32)
    Wv = np.asarray(inputs["Wv"], np.float32)
    Wo = np.asarray(inputs["Wo"], np.float32)
    mask = np.asarray(inputs["attention_mask"], np.float32)[0, 0]

    # hT [D, T] -> [128, NT, NDB, TT]: partition p, tile it line contiguous
    hT = h.reshape(T, D).T.astype(BF16)                  # [D, T]
    hTP = np.ascontiguousarray(
        hT.reshape(NDB, 128, NT, TT).transpose(1, 2, 0, 3).reshape(128, -1)
    )

    cosT = cos.T                                      # [64, S]
    sinT = sin.T
    cosD = np.ascontiguousarray(cosT.astype(BF16))
    s_signed = np.vstack([-sinT[0:32], sinT[32:64]])  # rot_half sign baked in
    # pre-swap so that z[p] = x[p]*sinp[p]; m2[p] = z[swap(p)] equals
    # rot_half(x)[p] * sin_signed[p]  (swap = 32-row block pairs 0<->1;
    # rows 64:128 and batch 1 are expanded on-device)
    swap_idx = np.concatenate([np.arange(32, 64), np.arange(0, 32)])
    sinD = np.ascontiguousarray(s_signed[swap_idx].astype(BF16))

    maskT = np.ascontiguousarray(mask.T).astype(np.float32)

    def pack(w):
        # [R, C] with R = n*128 -> [128, n*C] so partition lines are
        # contiguous in DRAM (one fat DMA descriptor per partition)
        r, c = w.shape
        n = r // 128
        return np.ascontiguousarray(
            w.reshape(n, 128, c).transpose(1, 0, 2).reshape(128, n * c)
        ).astype(BF16)

    in_maps = []
    for g in range(8):
        wqT = Wq[g * EQ : (g + 1) * EQ].T              # [D, EQ]
        wkvT = np.concatenate(
            [Wk[g * HD : (g + 1) * HD].T, Wv[g * HD : (g + 1) * HD].T], axis=1
        )                                               # [D, 128]
        woT = Wo[:, g * EQ : (g + 1) * EQ].T            # [EQ, D]
        in_maps.append(
            {
                "hTP": hTP,
                "wqP": pack(wqT),
                "wkvP": pack(wkvT),
                "woP": pack(woT),
                "cosD": cosD,
                "sinD": sinD,
                "maskT": maskT,
            }
        )
    return in_maps


def _is_causal(mask):
    m = np.asarray(mask, np.float32)[0, 0]
    tri = np.tril(np.ones((S, S), bool))
    return bool(np.all(m[tri] == 0.0) and np.all(m[~tri] <= -1e8))


def _assemble(results):
    # outP [128, NT, NDB, TT] -> [D, T]; outP2 [128, NDB, QT] adds into
    # the last query tile's columns
    acc = np.zeros((128, NT, NDB, TT), np.float32)
    for r in results:
        acc += r["outP"].astype(np.float32).reshape(128, NT, NDB, TT)
        acc[:, NT - 1] += r["outP2"].astype(np.float32).reshape(128, NDB, QT)
    out = acc.transpose(2, 0, 1, 3).reshape(D, T)   # [NDB*128=D, NT*TT=T]
    return np.ascontiguousarray(out.reshape(D, B, S).transpose(1, 2, 0))


def kernel(**inputs) -> np.ndarray:
    from concourse.bass_utils import run_bass_kernel_spmd

    causal = _is_causal(inputs["attention_mask"])
    key = ("prog", causal)
    if key not in _CACHE:
        _CACHE[key] = _build_program(causal)
    nc = _CACHE[key]

    in_maps = _host_inputs(inputs, causal)
    res = run_bass_kernel_spmd(nc, in_maps, core_ids=list(range(8)))
    return _assemble(res.results)
